# revision 56
# baseline (speedup 1.0000x reference)
"""Bidirectional Mamba layer for Trainium2 (8 NeuronCores).

Sharding: core = (batch b in {0,1}) x (direction in {fwd,bwd}) x (d_inner half).
All 8 cores run one SPMD program with per-core input arrays; no collectives.
The host flips the sequence for the backward direction, permutes u-channels so
each core's own d_inner half is channel-tiles 0..5, and pre-builds every weight
layout (including the depthwise-conv taps and the D-skip as ready diagonal
matrices) so the engines never build operands at runtime.

v2: the sequence is processed in two 512-column chunks, software-pipelined so
the selective scan for chunk 0 runs while the tensor engine projects chunk 1.
Engine assignment per (d-tile, state-group): dA=exp(delta*A) on ACT, dbu and
the hardware tensor_tensor_scan on DVE (bf16 keeps dbu in the 2x DVE mode),
g = h*C mostly on the otherwise-idle GPSIMD engine, y = sum_n g as identity
matmuls accumulated in PSUM on PE, gating on DVE. Chunk-1 scans chain the
chunk-0 state via per-(d,n) carry columns and the scan's initial-AP operand.
"""
import sys

sys.path.insert(0, "/opt/trn_rl_repo")

from contextlib import ExitStack

import ml_dtypes
import numpy as np

import concourse.bass as bass
import concourse.mybir as mybir
import concourse.tile as tile
from concourse import bacc
from concourse.bass_utils import run_bass_kernel_spmd

D_MODEL = 768
D_STATE = 16
D_INNER = 1536
DT_RANK = 48
D_CONV = 4
BATCH = 2
SEQ = 1024
DH = D_INNER // 2          # 768 scan channels per core
P = 128
KM = D_MODEL // P          # 6 k-tiles over d_model
MU = D_INNER // P          # 12 m-tiles for full u
MH = DH // P               # 6 m-tiles for the own half
CH = 512                   # chunk width (2 chunks over SEQ)
NB = 4                     # states per scan group
NGRP = D_STATE // NB       # 4 groups
SP = CH + 2                # scan block width incl 2 zero/pad columns
SETS = ((0, 1, 2), (3, 4, 5))

F32 = mybir.dt.float32
F32R = mybir.dt.float32r
BF16 = mybir.dt.bfloat16
AF = mybir.ActivationFunctionType
OP = mybir.AluOpType

_CACHE = {}


def _build():
    nc = bacc.Bacc("TRN2", target_bir_lowering=False, debug=False)

    xT = nc.dram_tensor("xT", [P, KM, SEQ], F32R, kind="ExternalInput")
    wuX = nc.dram_tensor("wuX", [MU, P, KM * P], F32R, kind="ExternalInput")
    wzX = nc.dram_tensor("wzX", [MH, P, KM * P], F32R, kind="ExternalInput")
    djX = nc.dram_tensor("djX", [MU, P, D_CONV * P], BF16, kind="ExternalInput")
    dDX = nc.dram_tensor("dDX", [P, MH * P], BF16, kind="ExternalInput")
    eyeX = nc.dram_tensor("eyeX", [P, P], BF16, kind="ExternalInput")
    cbias = nc.dram_tensor("cbias", [P, MU], F32, kind="ExternalInput")
    xpX = nc.dram_tensor("xpX", [P, MU, 80], BF16, kind="ExternalInput")
    dtwT = nc.dram_tensor("dtwT", [DT_RANK + 1, DH], F32R, kind="ExternalInput")
    ones1 = nc.dram_tensor("ones1", [1, CH], F32R, kind="ExternalInput")
    Amat = nc.dram_tensor("Amat", [P, MH, D_STATE], F32, kind="ExternalInput")
    owX = nc.dram_tensor("owX", [P, KM, MH * P], BF16, kind="ExternalInput")
    outp = nc.dram_tensor("outp", [D_MODEL, SEQ], F32, kind="ExternalOutput")

    with tile.TileContext(nc) as tc, ExitStack() as top:
        persist = top.enter_context(tc.tile_pool(name="persist", bufs=1))
        xs_pool = top.enter_context(tc.tile_pool(name="xs", bufs=1))
        uoth_pool = top.enter_context(tc.tile_pool(name="uoth", bufs=6))
        wpool = top.enter_context(tc.tile_pool(name="wst", bufs=2))
        djpool = top.enter_context(tc.tile_pool(name="djst", bufs=2))
        ubuf_pool = top.enter_context(tc.tile_pool(name="ubuf", bufs=2))
        xdr_pool = top.enter_context(tc.tile_pool(name="xdr", bufs=2))
        xbc_pool = top.enter_context(tc.tile_pool(name="xbc", bufs=2))
        bcg_pool = top.enter_context(tc.tile_pool(name="bcg", bufs=2))
        da_pool = top.enter_context(tc.tile_pool(name="da", bufs=4))
        dbu_pool = top.enter_context(tc.tile_pool(name="dbu", bufs=4))
        h_pool = top.enter_context(tc.tile_pool(name="h", bufs=5))
        g_pool = top.enter_context(tc.tile_pool(name="g", bufs=5))
        yf_pool = top.enter_context(tc.tile_pool(name="yf", bufs=1))
        ot_pool = top.enter_context(tc.tile_pool(name="ot", bufs=2))
        ow_pool = top.enter_context(tc.tile_pool(name="owst", bufs=2))
        dram = top.enter_context(tc.tile_pool(name="dram", bufs=2, space="DRAM"))
        ps_a = top.enter_context(tc.tile_pool(name="ps_a", bufs=3, space="PSUM"))
        ps_xg = top.enter_context(tc.tile_pool(name="ps_xg", bufs=2, space="PSUM"))
        ps_y = top.enter_context(tc.tile_pool(name="ps_y", bufs=3, space="PSUM"))

        u_own = persist.tile([P, MH, SEQ], BF16, tag="uown")
        sz = persist.tile([P, MH, SEQ], BF16, tag="sz")
        delta = persist.tile([P, MH, SEQ], BF16, tag="dl")
        wdu = persist.tile([P, MH, SEQ], BF16, tag="wdu")
        carry = persist.tile([P, MH, D_STATE], BF16, tag="carry")
        A_sb = persist.tile([P, MH, D_STATE], F32, tag="A")
        cb_sb = persist.tile([P, MU], F32, tag="cb")
        dtw_sb = persist.tile([DT_RANK + 1, DH], F32R, tag="dtw")
        eye_sb = persist.tile([P, P], BF16, tag="eye")
        dD_sb = persist.tile([P, MH * P], BF16, tag="dD")
        xp_sb = persist.tile([P, MU, 80], BF16, tag="xp")
        halo = persist.tile([P, MU, 3], BF16, tag="halo")
        token = persist.tile([P, 1], BF16, tag="tok")
        one3 = persist.tile([P, 3], BF16, tag="one3")
        xs = xs_pool.tile([P, KM, SEQ], F32R, tag="xs")

        # first chunk of x + the first weight tiles lead the DMA queue so the
        # tensor engine starts as early as possible; bulk loads follow later
        nc.sync.dma_start(out=xs[:, :, 0:CH], in_=xT[:, :, 0:CH])
        nc.sync.dma_start(out=cb_sb, in_=cbias[:, :])
        nc.gpsimd.memset(one3, 1.0)
        nc.sync.dma_start(out=xp_sb, in_=xpX[:, :, :])

        state = {"ubuf_n": 0, "da_n": 0, "dbu_n": 0,
                 "uref": {}, "psx": {}, "yps": {}, "bcd": {}, "yf": {}}

        def cols(th):
            return slice(th * CH, (th + 1) * CH)

        # ---------------- phase A building blocks ----------------
        def psx_tile(name):
            t = ps_xg.tile([P, CH], F32, tag="pg", name=name)
            return t[0:80, :]

        def u_inproj(th, m):
            wu_m = wpool.tile([P, KM * P], F32R, tag="w")
            nc.sync.dma_start(out=wu_m, in_=wuX[m, :, :])
            dj = djpool.tile([P, D_CONV * P], BF16, tag="dj")
            nc.sync.dma_start(out=dj, in_=djX[m, :, :])
            ps = ps_a.tile([P, CH], F32, tag="ps")
            for k in range(KM):
                nc.tensor.matmul(ps, wu_m[:, k * P:(k + 1) * P],
                                 xs[:, k, cols(th)],
                                 start=(k == 0), stop=(k == KM - 1))
            return ps, dj

        def u_block(th, m, defer, ps, dj):
            """causal conv -> (silu or deferred) u tile, plus the xproj
            contribution when not deferred."""
            ub = ubuf_pool.tile([P, 3 + CH], BF16, tag="ub")
            if th == 0:
                if state["ubuf_n"] < 2:
                    nc.gpsimd.memset(ub[:, 0:3], 0.0)
                state["ubuf_n"] += 1
            else:
                nc.gpsimd.tensor_tensor(out=ub[:, 0:3], in0=halo[:, m, :],
                                        in1=one3, op=OP.mult)
            if th == 0:
                nc.scalar.copy(out=ub[:, 3:3 + CH], in_=ps)
                nc.gpsimd.tensor_tensor(out=halo[:, m, :], in0=ub[:, CH:CH + 3],
                                        in1=one3, op=OP.mult)
            else:
                # chunk-1 staging on DVE: lands in the scan-stream troughs and
                # unloads the oversubscribed ACT transition window
                nc.vector.tensor_scalar_mul(ub[:, 3:3 + CH], ps, 1.0)
            psc = ps_a.tile([P, CH], F32, tag="ps")
            for j in range(D_CONV):
                nc.tensor.matmul(psc, dj[:, j * P:(j + 1) * P],
                                 ub[:, j:j + CH],
                                 start=(j == 0), stop=(j == D_CONV - 1))
            if m < MH:
                dest = u_own[:, m, cols(th)]
            else:
                dest = uoth_pool.tile([P, CH], BF16, tag="uo", name=f"uo{th}_{m}")
            if not defer:
                nc.scalar.activation(out=dest, in_=psc, func=AF.Silu,
                                     bias=cb_sb[:, m:m + 1])
                nc.tensor.matmul(state["psx"][th], xp_sb[:, m, :], dest,
                                 start=(m == 0), stop=(m == MU - 1))
            else:
                nc.scalar.activation(out=dest, in_=psc, func=AF.Identity,
                                     bias=cb_sb[:, m:m + 1])
            state["uref"][(th, m)] = dest

        def z_block(th, mz, defer):
            wz_m = wpool.tile([P, KM * P], F32R, tag="w")
            nc.sync.dma_start(out=wz_m, in_=wzX[mz, :, :])
            ps = ps_a.tile([P, CH], F32, tag="ps")
            for k in range(KM):
                nc.tensor.matmul(ps, wz_m[:, k * P:(k + 1) * P],
                                 xs[:, k, cols(th)],
                                 start=(k == 0), stop=(k == KM - 1))
            if not defer:
                nc.scalar.activation(out=sz[:, mz, cols(th)], in_=ps, func=AF.Silu)
            else:
                nc.vector.tensor_scalar_mul(sz[:, mz, cols(th)], ps, 1.0)

        def silu_batch(th):
            """Deferred in-place silus for chunk th (u own, u other, z).
            The zero `token` bias is a scheduling fence: the greedy per-engine
            scheduler would otherwise hoist these silus into idle slots of the
            chunk-0 dA exp stream, thrashing the ACT function table (silu and
            exp share no table). The token is produced only after the last
            chunk-0 dA tile, so these stay one contiguous batch."""
            for m in range(MU):
                dest = state["uref"][(th, m)]
                nc.scalar.activation(out=dest, in_=dest, func=AF.Silu,
                                     bias=token[:, 0:1])
            for mz in range(MH):
                s = sz[:, mz, cols(th)]
                nc.scalar.activation(out=s, in_=s, func=AF.Silu,
                                     bias=token[:, 0:1])

        def xproj_late(th):
            for m in range(MU):
                nc.tensor.matmul(state["psx"][th], xp_sb[:, m, :],
                                 state["uref"][(th, m)],
                                 start=(m == 0), stop=(m == MU - 1))

        def dt_softplus(th):
            psx = state["psx"][th]
            xdr = xdr_pool.tile([64, CH], F32R, tag="xdr")
            nc.scalar.copy(out=xdr[0:32, :], in_=psx[0:32, :])
            nc.scalar.copy(out=xdr[32:64, :], in_=psx[32:64, :])
            nc.scalar.dma_start(out=xdr[DT_RANK:DT_RANK + 1, :],
                                in_=ones1[:, :])
            dcol = delta[:, :, cols(th)]
            for m in range(MH):
                psd = ps_a.tile([P, CH], F32, tag="ps")
                nc.tensor.matmul(psd, dtw_sb[:, m * P:(m + 1) * P],
                                 xdr[0:DT_RANK + 1, :], start=True, stop=True)
                nc.scalar.activation(out=delta[:, m, cols(th)], in_=psd,
                                     func=AF.Exp)
            # softplus tail: delta = ln(exp(.) + 1), computed in place
            nc.scalar.activation(out=dcol, in_=dcol, func=AF.Ln, bias=1.0)

        def bc_stage(th):
            psx = state["psx"][th]
            xbc = xbc_pool.tile([48, CH], BF16, tag="xbc")
            nc.scalar.copy(out=xbc[0:32, :], in_=psx[32:64, :])
            nc.scalar.copy(out=xbc[32:48, :], in_=psx[64:80, :])
            bcd = dram.tile([2 * D_STATE, CH], BF16, tag="bcd")
            nc.scalar.dma_start(out=bcd, in_=xbc[16:48, :])
            state["bcd"][th] = bcd

        def w_mult(th):
            for m in range(MH):
                nc.vector.tensor_tensor(out=wdu[:, m, cols(th)],
                                        in0=delta[:, m, cols(th)],
                                        in1=u_own[:, m, cols(th)], op=OP.mult)

        # ---------------- phase B: scans ----------------
        def dA_set(th, s):
            """dA for one d-tile set. Groups 0-1 (n=0..7) are exps on ACT;
            groups 2-3 reuse them as DVE bf16 products: q^(8+k) = q^8*q^k
            (A is the S4D-real init, so dA_n = exp(-(n+1)*delta) = q^(n+1)).
            The da pool holds a full set so product sources stay live."""
            for ng in range(NGRP):
                for m in SETS[s]:
                    dat = da_pool.tile([P, NB, SP], BF16, tag="da")
                    if state["da_n"] < 4:
                        nc.gpsimd.memset(dat[:, :, CH:SP], 0.0)
                    state["da_n"] += 1
                    for j in range(NB):
                        n = ng * NB + j
                        nc.scalar.activation(out=dat[:, j, 0:CH],
                                             in_=delta[:, m, cols(th)],
                                             func=AF.Exp,
                                             scale=A_sb[:, m, n:n + 1])
                    state[("da", th, s, ng, m)] = dat

        def scan_set(th, s):
            """One set of 3 d-tiles: all 4 state-groups, scans + g + yacc."""
            gt_ref = {}
            yps = {m: ps_y.tile([P, CH], F32, tag="yps", name=f"yps{th}{s}{m}")
                   for m in SETS[s]}
            state["yps"].update({(th, m): yps[m] for m in SETS[s]})
            for ng in range(NGRP):
                bcgt = bcg_pool.tile([P, 2, NB, CH], BF16, tag="bcg")
                src = bass.AP(
                    tensor=state["bcd"][th].tensor,
                    offset=state["bcd"][th].offset + ng * NB * CH,
                    ap=[[0, P], [D_STATE * CH, 2], [CH, NB], [1, CH]])
                nc.scalar.dma_start(out=bcgt, in_=src)
                for m in SETS[s]:
                    dat = state[("da", th, s, ng, m)]
                    dbut = dbu_pool.tile([P, NB, SP], BF16, tag="dbu")
                    if state["dbu_n"] < 4:
                        nc.gpsimd.memset(dbut[:, :, CH:SP], 0.0)
                    state["dbu_n"] += 1
                    nc.vector.tensor_tensor(
                        out=dbut[:, :, 0:CH],
                        in0=wdu[:, m, cols(th)].unsqueeze(1)
                            .broadcast_to([P, NB, CH]),
                        in1=bcgt[:, 0, :, :], op=OP.mult)
                    ht = h_pool.tile([P, NB, SP], BF16, tag="h")
                    if th == 0:
                        nc.vector.tensor_tensor_scan(
                            out=ht.rearrange("p a b -> p (a b)"),
                            data0=dat.rearrange("p a b -> p (a b)"),
                            data1=dbut.rearrange("p a b -> p (a b)"),
                            initial=0.0, op0=OP.mult, op1=OP.add)
                        nc.vector.tensor_scalar_mul(
                            carry[:, m, ng * NB:(ng + 1) * NB],
                            ht[:, :, CH - 1:CH].rearrange("p a b -> p (a b)"),
                            1.0)
                    else:
                        for j in range(NB):
                            n = ng * NB + j
                            nc.vector.tensor_tensor_scan(
                                out=ht[:, j, 0:CH], data0=dat[:, j, 0:CH],
                                data1=dbut[:, j, 0:CH],
                                initial=carry[:, m, n:n + 1],
                                op0=OP.mult, op1=OP.add)
                    gt = g_pool.tile([P, NB, CH], BF16, tag="g")
                    # g = h*C split 3:1 between GPSIMD and DVE so neither
                    # paces the chunk pipeline alone
                    nc.gpsimd.tensor_tensor(out=gt[:, 0:3, :],
                                            in0=ht[:, 0:3, 0:CH],
                                            in1=bcgt[:, 1, 0:3, :], op=OP.mult)
                    nc.vector.tensor_tensor(out=gt[:, 3, :],
                                            in0=ht[:, 3, 0:CH],
                                            in1=bcgt[:, 1, 3, :], op=OP.mult)
                    gt_ref[(m, ng)] = gt
                for m in SETS[s]:
                    for j in range(NB):
                        nc.tensor.matmul(yps[m][:, :], eye_sb,
                                         gt_ref[(m, ng)][:, j, :],
                                         start=(ng == 0 and j == 0), stop=False)
            for m in SETS[s]:
                nc.tensor.matmul(yps[m][:, :], dD_sb[:, m * P:(m + 1) * P],
                                 u_own[:, m, cols(th)], start=False, stop=True)

        def yf_tile(th):
            yft = state["yf"].get(th)
            if yft is None:
                yft = yf_pool.tile([P, MH, CH], BF16, tag="yf", name=f"yf{th}")
                state["yf"][th] = yft
            return yft

        def gates(th, s):
            yft = yf_tile(th)
            for m in SETS[s]:
                nc.vector.tensor_tensor(out=yft[:, m, :],
                                        in0=state["yps"][(th, m)],
                                        in1=sz[:, m, cols(th)], op=OP.mult)

        def out_proj(th):
            yft = state["yf"][th]
            for mo in range(KM):
                owt = ow_pool.tile([P, MH * P], BF16, tag="ow")
                nc.sync.dma_start(out=owt, in_=owX[:, mo, :])
                psg = ps_xg.tile([P, CH], F32, tag="pg")
                for k in range(MH):
                    nc.tensor.matmul(psg, owt[:, k * P:(k + 1) * P],
                                     yft[:, k, :],
                                     start=(k == 0), stop=(k == MH - 1))
                ot = ot_pool.tile([P, CH], F32, tag="ot")
                nc.scalar.copy(out=ot, in_=psg)
                nc.sync.dma_start(out=outp[mo * P:(mo + 1) * P, cols(th)],
                                  in_=ot)

        def out_proj_stream(th):
            """Chunk-1 out_proj: per-set streamed accumulation. Six psg banks
            (4 from ps_a, idle after phase A, + 2 from ps_xg) accumulate the
            k-contractions as each gate set completes, so only one matmul per
            output tile trails the final gate."""
            yft = yf_tile(th)
            NS = 5  # five tiles streamed (3 ps_a + 2 ps_xg banks); the last
            ows, psgs = [], []
            for mo in range(NS):
                owt = ow_pool.tile([P, MH * P], BF16, tag="ow",
                                   name=f"owS{mo}")
                nc.sync.dma_start(out=owt, in_=owX[:, mo, :])
                pool = ps_a if mo < 3 else ps_xg
                tag = "ps" if mo < 3 else "pg"
                psgs.append(pool.tile([P, CH], F32, tag=tag, name=f"psg{mo}"))
                ows.append(owt)
            for s in range(len(SETS)):
                yield s
                for mo in range(NS):
                    for k in SETS[s]:
                        nc.tensor.matmul(psgs[mo][:, :],
                                         ows[mo][:, k * P:(k + 1) * P],
                                         yft[:, k, :],
                                         start=(k == 0), stop=(k == MH - 1))
            for mo in range(NS):
                ot = ot_pool.tile([P, CH], F32, tag="ot")
                nc.scalar.copy(out=ot, in_=psgs[mo])
                nc.sync.dma_start(out=outp[mo * P:(mo + 1) * P, cols(th)],
                                  in_=ot)
            for mo in range(NS, KM):
                owt = ow_pool.tile([P, MH * P], BF16, tag="ow")
                nc.sync.dma_start(out=owt, in_=owX[:, mo, :])
                psg = ps_a.tile([P, CH], F32, tag="ps")
                for k in range(MH):
                    nc.tensor.matmul(psg, owt[:, k * P:(k + 1) * P],
                                     yft[:, k, :],
                                     start=(k == 0), stop=(k == MH - 1))
                ot = ot_pool.tile([P, CH], F32, tag="ot")
                nc.scalar.copy(out=ot, in_=psg)
                nc.sync.dma_start(out=outp[mo * P:(mo + 1) * P, cols(th)],
                                  in_=ot)

        # ---------------- emission schedule ----------------
        state["psx"][0] = psx_tile("psx0")
        pend = None
        for m in range(MU):
            cur = (0, m, u_inproj(0, m))
            if pend is not None:
                (pth, pm, (pps, pdj)) = pend
                u_block(pth, pm, False, pps, pdj)
            pend = cur
            if m == 1:
                nc.sync.dma_start(out=dtw_sb, in_=dtwT[:, :])
                nc.sync.dma_start(out=A_sb, in_=Amat[:, :, :])
        (pth, pm, (pps, pdj)) = pend
        u_block(pth, pm, False, pps, pdj)
        for mz in range(MH):
            z_block(0, mz, defer=False)
            if mz == 0:
                nc.sync.dma_start(out=xs[:, :, CH:SEQ], in_=xT[:, :, CH:SEQ])
            elif mz == 2:
                nc.sync.dma_start(out=eye_sb, in_=eyeX[:, :])
                nc.sync.dma_start(out=dD_sb, in_=dDX[:, :])
        dt_softplus(0)
        bc_stage(0)
        w_mult(0)

        # chunk-1 projections (pre-silu) — PE/ACT-copy work that overlaps
        # the chunk-0 scan stream below
        pend = None
        for m in range(MU):
            cur = (1, m, u_inproj(1, m))
            if pend is not None:
                (pth, pm, (pps, pdj)) = pend
                u_block(pth, pm, True, pps, pdj)
            pend = cur
        (pth, pm, (pps, pdj)) = pend
        u_block(pth, pm, True, pps, pdj)
        for mz in range(MH):
            z_block(1, mz, defer=True)

        dA_set(0, 0)
        scan_set(0, 0)
        # scheduling fence: token is written once the first set of chunk-0
        # dA tiles exists, releasing the deferred silu batch below after the
        # first contiguous block of exps
        last_da = state[("da", 0, 0, NGRP - 1, SETS[0][-1])]
        nc.vector.tensor_scalar_mul(token, last_da[:, 0, 0:1], 0.0)

        silu_batch(1)
        dA_set(0, 1)
        state["psx"][1] = psx_tile("psx1")
        xproj_late(1)
        dt_softplus(1)
        bc_stage(1)

        scan_set(0, 1)
        w_mult(1)
        gates(0, 0)
        gates(0, 1)

        dA_set(1, 0)
        out_proj(0)
        scan_set(1, 0)
        ops = out_proj_stream(1)
        next(ops)
        dA_set(1, 1)
        scan_set(1, 1)
        gates(1, 0)
        next(ops)
        gates(1, 1)
        for _ in ops:
            pass

    nc.finalize()
    return nc


def _prep_core(x, prm, b, direction, half):
    """Build the per-core input map. prm maps param name -> array."""
    xb = np.ascontiguousarray(x[b])                # (L, D_MODEL)
    if direction == 1:
        xb = np.ascontiguousarray(xb[::-1])
    in_w = prm["in_w"]
    conv_w = prm["conv_w"]
    conv_b = prm["conv_b"]
    xproj_w = prm["xproj_w"]
    dt_w = prm["dt_w"]
    dt_b = prm["dt_b"]
    Alog = prm["Alog"]
    Dp = prm["D"]
    out_w = prm["out_w"]

    own = np.arange(half * DH, (half + 1) * DH)
    oth = np.arange((1 - half) * DH, (2 - half) * DH)
    perm = np.concatenate([own, oth])              # u-channel permutation

    wu = in_w[0:D_INNER][perm]                     # (1536, 768), own half first
    wz = in_w[D_INNER:2 * D_INNER][own]            # (768, 768)
    cw = conv_w[perm]                              # (1536, 4)
    A = -np.exp(Alog[own])                         # (768, 16)
    bf = ml_dtypes.bfloat16

    def lhs_tiles(mat_t, kk, mm):
        # (K*P, M*P) -> (mm, P, kk*P): per m-tile, partition-contiguous rows
        return np.ascontiguousarray(
            mat_t.reshape(kk, P, mm, P).transpose(2, 1, 0, 3).reshape(mm, P, kk * P))

    # conv taps as diagonal matmul weights: djX[m, p, j*P+q] = (p==q)*cw[mP+p, j]
    eye = np.eye(P, dtype=np.float32)
    dj = np.einsum("pq,mpj->mpjq", eye,
                   cw.reshape(MU, P, D_CONV)).reshape(MU, P, D_CONV * P)
    # D-skip diagonals: dDX[p, k*P+q] = (p==q)*D[kP+p]
    dD = np.einsum("pq,kp->pkq", eye,
                   Dp[own].reshape(MH, P)).reshape(P, MH * P)
    # out_proj: owX[p, mo, k*P+q] = out_w[mo*P+q, own[k*P+p]]
    ow = out_w[:, own].reshape(KM, P, MH, P).transpose(3, 0, 2, 1)  # p,mo,k,q
    ow = np.ascontiguousarray(ow.transpose(0, 1, 2, 3)).reshape(P, KM, MH * P)

    return {
        "xT": np.ascontiguousarray(xb.T.reshape(KM, P, SEQ).transpose(1, 0, 2)),
        "wuX": lhs_tiles(wu.T, KM, MU),
        "wzX": lhs_tiles(wz.T, KM, MH),
        "djX": dj.astype(bf),
        "dDX": dD.astype(bf),
        "eyeX": eye.astype(bf),
        "cbias": np.ascontiguousarray(conv_b[perm].reshape(MU, P).T),
        "xpX": np.ascontiguousarray(
            xproj_w[:, perm].T.reshape(MU, P, 80).transpose(1, 0, 2)).astype(bf),
        "dtwT": np.ascontiguousarray(
            np.vstack([dt_w[own].T, dt_b[own][None, :]])),
        "ones1": np.ones((1, CH), dtype=np.float32),
        "Amat": np.ascontiguousarray(A.reshape(MH, P, D_STATE).transpose(1, 0, 2)),
        "owX": np.ascontiguousarray(ow).astype(bf),
    }


def _in_maps(inputs):
    x = inputs["x"]
    maps = []
    for b in range(BATCH):
        for direction in range(2):
            pfx = "f" if direction == 0 else "b"
            prm = {k: inputs[f"{pfx}_{k}"] for k in
                   ("in_w", "conv_w", "conv_b", "xproj_w", "dt_w", "dt_b",
                    "Alog", "D", "out_w")}
            for half in range(2):
                maps.append(_prep_core(x, prm, b, direction, half))
    return maps


def kernel(**inputs):
    inputs = {k: np.asarray(v, dtype=np.float32) for k, v in inputs.items()}
    nc = _CACHE.get("nc")
    if nc is None:
        nc = _build()
        _CACHE["nc"] = nc
    maps = _in_maps(inputs)
    res = run_bass_kernel_spmd(nc, maps, list(range(8)),
                               **_CACHE.get("run_kwargs", {}))
    _CACHE["last_results"] = res
    out = np.zeros((BATCH, SEQ, D_MODEL), dtype=np.float32)
    ci = 0
    for b in range(BATCH):
        for direction in range(2):
            for half in range(2):
                part = res.results[ci]["outp"].T          # (SEQ, D_MODEL)
                if direction == 1:
                    part = part[::-1]
                out[b] += part
                ci += 1
    return out


# revision 58
# speedup vs baseline: 1.0013x; 1.0013x over previous
"""Bidirectional Mamba layer for Trainium2 (8 NeuronCores).

Sharding: core = (batch b in {0,1}) x (direction in {fwd,bwd}) x (d_inner half).
All 8 cores run one SPMD program with per-core input arrays; no collectives.
The host flips the sequence for the backward direction, permutes u-channels so
each core's own d_inner half is channel-tiles 0..5, and pre-builds every weight
layout (including the depthwise-conv taps and the D-skip as ready diagonal
matrices) so the engines never build operands at runtime.

v2: the sequence is processed in two 512-column chunks, software-pipelined so
the selective scan for chunk 0 runs while the tensor engine projects chunk 1.
Engine assignment per (d-tile, state-group): dA=exp(delta*A) on ACT, dbu and
the hardware tensor_tensor_scan on DVE (bf16 keeps dbu in the 2x DVE mode),
g = h*C mostly on the otherwise-idle GPSIMD engine, y = sum_n g as identity
matmuls accumulated in PSUM on PE, gating on DVE. Chunk-1 scans chain the
chunk-0 state via per-(d,n) carry columns and the scan's initial-AP operand.
"""
import sys

sys.path.insert(0, "/opt/trn_rl_repo")

from contextlib import ExitStack

import ml_dtypes
import numpy as np

import concourse.bass as bass
import concourse.mybir as mybir
import concourse.tile as tile
from concourse import bacc
from concourse.bass_utils import run_bass_kernel_spmd

D_MODEL = 768
D_STATE = 16
D_INNER = 1536
DT_RANK = 48
D_CONV = 4
BATCH = 2
SEQ = 1024
DH = D_INNER // 2          # 768 scan channels per core
P = 128
KM = D_MODEL // P          # 6 k-tiles over d_model
MU = D_INNER // P          # 12 m-tiles for full u
MH = DH // P               # 6 m-tiles for the own half
CH = 512                   # chunk width (2 chunks over SEQ)
NB = 4                     # states per scan group
NGRP = D_STATE // NB       # 4 groups
SP = CH + 2                # scan block width incl 2 zero/pad columns
SETS = ((0, 1, 2), (3, 4, 5))

F32 = mybir.dt.float32
F32R = mybir.dt.float32r
BF16 = mybir.dt.bfloat16
AF = mybir.ActivationFunctionType
OP = mybir.AluOpType

_CACHE = {}


def _build():
    nc = bacc.Bacc("TRN2", target_bir_lowering=False, debug=False)

    xT = nc.dram_tensor("xT", [P, KM, SEQ], F32R, kind="ExternalInput")
    wuX = nc.dram_tensor("wuX", [MU, P, KM * P], F32R, kind="ExternalInput")
    wzX = nc.dram_tensor("wzX", [MH, P, KM * P], F32R, kind="ExternalInput")
    djX = nc.dram_tensor("djX", [MU, P, D_CONV * P], BF16, kind="ExternalInput")
    dDX = nc.dram_tensor("dDX", [P, MH * P], BF16, kind="ExternalInput")
    eyeX = nc.dram_tensor("eyeX", [P, P], BF16, kind="ExternalInput")
    cbias = nc.dram_tensor("cbias", [P, MU], F32, kind="ExternalInput")
    xpX = nc.dram_tensor("xpX", [P, MU, 80], BF16, kind="ExternalInput")
    dtwT = nc.dram_tensor("dtwT", [DT_RANK + 1, DH], F32R, kind="ExternalInput")
    ones1 = nc.dram_tensor("ones1", [1, CH], F32R, kind="ExternalInput")
    Amat = nc.dram_tensor("Amat", [P, MH, D_STATE], F32, kind="ExternalInput")
    owX = nc.dram_tensor("owX", [P, KM, MH * P], BF16, kind="ExternalInput")
    outp = nc.dram_tensor("outp", [D_MODEL, SEQ], F32, kind="ExternalOutput")

    with tile.TileContext(nc) as tc, ExitStack() as top:
        persist = top.enter_context(tc.tile_pool(name="persist", bufs=1))
        xs_pool = top.enter_context(tc.tile_pool(name="xs", bufs=1))
        uoth_pool = top.enter_context(tc.tile_pool(name="uoth", bufs=6))
        wpool = top.enter_context(tc.tile_pool(name="wst", bufs=2))
        djpool = top.enter_context(tc.tile_pool(name="djst", bufs=2))
        ubuf_pool = top.enter_context(tc.tile_pool(name="ubuf", bufs=3))
        xdr_pool = top.enter_context(tc.tile_pool(name="xdr", bufs=2))
        xbc_pool = top.enter_context(tc.tile_pool(name="xbc", bufs=2))
        bcg_pool = top.enter_context(tc.tile_pool(name="bcg", bufs=2))
        da_pool = top.enter_context(tc.tile_pool(name="da", bufs=4))
        dbu_pool = top.enter_context(tc.tile_pool(name="dbu", bufs=4))
        h_pool = top.enter_context(tc.tile_pool(name="h", bufs=5))
        g_pool = top.enter_context(tc.tile_pool(name="g", bufs=5))
        yf_pool = top.enter_context(tc.tile_pool(name="yf", bufs=1))
        ot_pool = top.enter_context(tc.tile_pool(name="ot", bufs=2))
        ow_pool = top.enter_context(tc.tile_pool(name="owst", bufs=2))
        dram = top.enter_context(tc.tile_pool(name="dram", bufs=2, space="DRAM"))
        ps_a = top.enter_context(tc.tile_pool(name="ps_a", bufs=3, space="PSUM"))
        ps_xg = top.enter_context(tc.tile_pool(name="ps_xg", bufs=2, space="PSUM"))
        ps_y = top.enter_context(tc.tile_pool(name="ps_y", bufs=3, space="PSUM"))

        u_own = persist.tile([P, MH, SEQ], BF16, tag="uown")
        sz = persist.tile([P, MH, SEQ], BF16, tag="sz")
        delta = persist.tile([P, MH, SEQ], BF16, tag="dl")
        wdu = persist.tile([P, MH, SEQ], BF16, tag="wdu")
        carry = persist.tile([P, MH, D_STATE], BF16, tag="carry")
        A_sb = persist.tile([P, MH, D_STATE], F32, tag="A")
        cb_sb = persist.tile([P, MU], F32, tag="cb")
        dtw_sb = persist.tile([DT_RANK + 1, DH], F32R, tag="dtw")
        eye_sb = persist.tile([P, P], BF16, tag="eye")
        dD_sb = persist.tile([P, MH * P], BF16, tag="dD")
        xp_sb = persist.tile([P, MU, 80], BF16, tag="xp")
        halo = persist.tile([P, MU, 3], BF16, tag="halo")
        token = persist.tile([P, 1], BF16, tag="tok")
        one3 = persist.tile([P, 3], BF16, tag="one3")
        xs = xs_pool.tile([P, KM, SEQ], F32R, tag="xs")

        # first chunk of x + the first weight tiles lead the DMA queue so the
        # tensor engine starts as early as possible; bulk loads follow later
        nc.sync.dma_start(out=xs[:, :, 0:CH], in_=xT[:, :, 0:CH])
        nc.sync.dma_start(out=cb_sb, in_=cbias[:, :])
        nc.gpsimd.memset(one3, 1.0)
        nc.sync.dma_start(out=xp_sb, in_=xpX[:, :, :])

        state = {"ubuf_n": 0, "da_n": 0, "dbu_n": 0,
                 "uref": {}, "psx": {}, "yps": {}, "bcd": {}, "yf": {}}

        def cols(th):
            return slice(th * CH, (th + 1) * CH)

        # ---------------- phase A building blocks ----------------
        def psx_tile(name):
            t = ps_xg.tile([P, CH], F32, tag="pg", name=name)
            return t[0:80, :]

        def u_inproj(th, m):
            wu_m = wpool.tile([P, KM * P], F32R, tag="w")
            nc.sync.dma_start(out=wu_m, in_=wuX[m, :, :])
            dj = djpool.tile([P, D_CONV * P], BF16, tag="dj")
            nc.sync.dma_start(out=dj, in_=djX[m, :, :])
            ps = ps_a.tile([P, CH], F32, tag="ps")
            for k in range(KM):
                nc.tensor.matmul(ps, wu_m[:, k * P:(k + 1) * P],
                                 xs[:, k, cols(th)],
                                 start=(k == 0), stop=(k == KM - 1))
            return ps, dj

        def u_block(th, m, defer, ps, dj):
            """causal conv -> (silu or deferred) u tile, plus the xproj
            contribution when not deferred."""
            ub = ubuf_pool.tile([P, 3 + CH], BF16, tag="ub")
            if th == 0:
                if state["ubuf_n"] < 3:
                    nc.gpsimd.memset(ub[:, 0:3], 0.0)
                state["ubuf_n"] += 1
            else:
                nc.gpsimd.tensor_tensor(out=ub[:, 0:3], in0=halo[:, m, :],
                                        in1=one3, op=OP.mult)
            if th == 0:
                nc.scalar.copy(out=ub[:, 3:3 + CH], in_=ps)
                nc.gpsimd.tensor_tensor(out=halo[:, m, :], in0=ub[:, CH:CH + 3],
                                        in1=one3, op=OP.mult)
            else:
                # chunk-1 staging on DVE: lands in the scan-stream troughs and
                # unloads the oversubscribed ACT transition window
                nc.vector.tensor_scalar_mul(ub[:, 3:3 + CH], ps, 1.0)
            psc = ps_a.tile([P, CH], F32, tag="ps")
            for j in range(D_CONV):
                nc.tensor.matmul(psc, dj[:, j * P:(j + 1) * P],
                                 ub[:, j:j + CH],
                                 start=(j == 0), stop=(j == D_CONV - 1))
            if m < MH:
                dest = u_own[:, m, cols(th)]
            else:
                dest = uoth_pool.tile([P, CH], BF16, tag="uo", name=f"uo{th}_{m}")
            if not defer:
                nc.scalar.activation(out=dest, in_=psc, func=AF.Silu,
                                     bias=cb_sb[:, m:m + 1])
                nc.tensor.matmul(state["psx"][th], xp_sb[:, m, :], dest,
                                 start=(m == 0), stop=(m == MU - 1))
            else:
                nc.scalar.activation(out=dest, in_=psc, func=AF.Identity,
                                     bias=cb_sb[:, m:m + 1])
            state["uref"][(th, m)] = dest

        def z_block(th, mz, defer):
            wz_m = wpool.tile([P, KM * P], F32R, tag="w")
            nc.sync.dma_start(out=wz_m, in_=wzX[mz, :, :])
            ps = ps_a.tile([P, CH], F32, tag="ps")
            for k in range(KM):
                nc.tensor.matmul(ps, wz_m[:, k * P:(k + 1) * P],
                                 xs[:, k, cols(th)],
                                 start=(k == 0), stop=(k == KM - 1))
            if not defer:
                nc.scalar.activation(out=sz[:, mz, cols(th)], in_=ps, func=AF.Silu)
            else:
                nc.vector.tensor_scalar_mul(sz[:, mz, cols(th)], ps, 1.0)

        def silu_batch(th):
            """Deferred in-place silus for chunk th (u own, u other, z).
            The zero `token` bias is a scheduling fence: the greedy per-engine
            scheduler would otherwise hoist these silus into idle slots of the
            chunk-0 dA exp stream, thrashing the ACT function table (silu and
            exp share no table). The token is produced only after the last
            chunk-0 dA tile, so these stay one contiguous batch."""
            for m in range(MU):
                dest = state["uref"][(th, m)]
                nc.scalar.activation(out=dest, in_=dest, func=AF.Silu,
                                     bias=token[:, 0:1])
            for mz in range(MH):
                s = sz[:, mz, cols(th)]
                nc.scalar.activation(out=s, in_=s, func=AF.Silu,
                                     bias=token[:, 0:1])

        def xproj_late(th):
            for m in range(MU):
                nc.tensor.matmul(state["psx"][th], xp_sb[:, m, :],
                                 state["uref"][(th, m)],
                                 start=(m == 0), stop=(m == MU - 1))

        def dt_softplus(th):
            psx = state["psx"][th]
            xdr = xdr_pool.tile([64, CH], F32R, tag="xdr")
            nc.scalar.copy(out=xdr[0:32, :], in_=psx[0:32, :])
            nc.scalar.copy(out=xdr[32:64, :], in_=psx[32:64, :])
            nc.scalar.dma_start(out=xdr[DT_RANK:DT_RANK + 1, :],
                                in_=ones1[:, :])
            dcol = delta[:, :, cols(th)]
            for m in range(MH):
                psd = ps_a.tile([P, CH], F32, tag="ps")
                nc.tensor.matmul(psd, dtw_sb[:, m * P:(m + 1) * P],
                                 xdr[0:DT_RANK + 1, :], start=True, stop=True)
                nc.scalar.activation(out=delta[:, m, cols(th)], in_=psd,
                                     func=AF.Exp)
            # softplus tail: delta = ln(exp(.) + 1), computed in place
            nc.scalar.activation(out=dcol, in_=dcol, func=AF.Ln, bias=1.0)

        def bc_stage(th):
            psx = state["psx"][th]
            xbc = xbc_pool.tile([48, CH], BF16, tag="xbc")
            nc.scalar.copy(out=xbc[0:32, :], in_=psx[32:64, :])
            nc.scalar.copy(out=xbc[32:48, :], in_=psx[64:80, :])
            bcd = dram.tile([2 * D_STATE, CH], BF16, tag="bcd")
            nc.scalar.dma_start(out=bcd, in_=xbc[16:48, :])
            state["bcd"][th] = bcd

        def w_mult(th):
            for m in range(MH):
                nc.vector.tensor_tensor(out=wdu[:, m, cols(th)],
                                        in0=delta[:, m, cols(th)],
                                        in1=u_own[:, m, cols(th)], op=OP.mult)

        # ---------------- phase B: scans ----------------
        def dA_set(th, s):
            """dA for one d-tile set. Groups 0-1 (n=0..7) are exps on ACT;
            groups 2-3 reuse them as DVE bf16 products: q^(8+k) = q^8*q^k
            (A is the S4D-real init, so dA_n = exp(-(n+1)*delta) = q^(n+1)).
            The da pool holds a full set so product sources stay live."""
            for ng in range(NGRP):
                for m in SETS[s]:
                    dat = da_pool.tile([P, NB, SP], BF16, tag="da")
                    if state["da_n"] < 4:
                        nc.gpsimd.memset(dat[:, :, CH:SP], 0.0)
                    state["da_n"] += 1
                    for j in range(NB):
                        n = ng * NB + j
                        nc.scalar.activation(out=dat[:, j, 0:CH],
                                             in_=delta[:, m, cols(th)],
                                             func=AF.Exp,
                                             scale=A_sb[:, m, n:n + 1])
                    state[("da", th, s, ng, m)] = dat

        def scan_set(th, s):
            """One set of 3 d-tiles: all 4 state-groups, scans + g + yacc."""
            gt_ref = {}
            yps = {m: ps_y.tile([P, CH], F32, tag="yps", name=f"yps{th}{s}{m}")
                   for m in SETS[s]}
            state["yps"].update({(th, m): yps[m] for m in SETS[s]})
            for ng in range(NGRP):
                bcgt = bcg_pool.tile([P, 2, NB, CH], BF16, tag="bcg")
                src = bass.AP(
                    tensor=state["bcd"][th].tensor,
                    offset=state["bcd"][th].offset + ng * NB * CH,
                    ap=[[0, P], [D_STATE * CH, 2], [CH, NB], [1, CH]])
                nc.scalar.dma_start(out=bcgt, in_=src)
                for m in SETS[s]:
                    dat = state[("da", th, s, ng, m)]
                    dbut = dbu_pool.tile([P, NB, SP], BF16, tag="dbu")
                    if state["dbu_n"] < 4:
                        nc.gpsimd.memset(dbut[:, :, CH:SP], 0.0)
                    state["dbu_n"] += 1
                    nc.vector.tensor_tensor(
                        out=dbut[:, :, 0:CH],
                        in0=wdu[:, m, cols(th)].unsqueeze(1)
                            .broadcast_to([P, NB, CH]),
                        in1=bcgt[:, 0, :, :], op=OP.mult)
                    ht = h_pool.tile([P, NB, SP], BF16, tag="h")
                    if th == 0:
                        nc.vector.tensor_tensor_scan(
                            out=ht.rearrange("p a b -> p (a b)"),
                            data0=dat.rearrange("p a b -> p (a b)"),
                            data1=dbut.rearrange("p a b -> p (a b)"),
                            initial=0.0, op0=OP.mult, op1=OP.add)
                        nc.vector.tensor_scalar_mul(
                            carry[:, m, ng * NB:(ng + 1) * NB],
                            ht[:, :, CH - 1:CH].rearrange("p a b -> p (a b)"),
                            1.0)
                    else:
                        for j in range(NB):
                            n = ng * NB + j
                            nc.vector.tensor_tensor_scan(
                                out=ht[:, j, 0:CH], data0=dat[:, j, 0:CH],
                                data1=dbut[:, j, 0:CH],
                                initial=carry[:, m, n:n + 1],
                                op0=OP.mult, op1=OP.add)
                    gt = g_pool.tile([P, NB, CH], BF16, tag="g")
                    # g = h*C split 3:1 between GPSIMD and DVE so neither
                    # paces the chunk pipeline alone
                    nc.gpsimd.tensor_tensor(out=gt[:, 0:3, :],
                                            in0=ht[:, 0:3, 0:CH],
                                            in1=bcgt[:, 1, 0:3, :], op=OP.mult)
                    nc.vector.tensor_tensor(out=gt[:, 3, :],
                                            in0=ht[:, 3, 0:CH],
                                            in1=bcgt[:, 1, 3, :], op=OP.mult)
                    gt_ref[(m, ng)] = gt
                for m in SETS[s]:
                    for j in range(NB):
                        nc.tensor.matmul(yps[m][:, :], eye_sb,
                                         gt_ref[(m, ng)][:, j, :],
                                         start=(ng == 0 and j == 0), stop=False)
            for m in SETS[s]:
                nc.tensor.matmul(yps[m][:, :], dD_sb[:, m * P:(m + 1) * P],
                                 u_own[:, m, cols(th)], start=False, stop=True)

        def yf_tile(th):
            yft = state["yf"].get(th)
            if yft is None:
                yft = yf_pool.tile([P, MH, CH], BF16, tag="yf", name=f"yf{th}")
                state["yf"][th] = yft
            return yft

        def gates(th, s):
            yft = yf_tile(th)
            for m in SETS[s]:
                nc.vector.tensor_tensor(out=yft[:, m, :],
                                        in0=state["yps"][(th, m)],
                                        in1=sz[:, m, cols(th)], op=OP.mult)

        def out_proj(th):
            yft = state["yf"][th]
            for mo in range(KM):
                owt = ow_pool.tile([P, MH * P], BF16, tag="ow")
                nc.sync.dma_start(out=owt, in_=owX[:, mo, :])
                psg = ps_xg.tile([P, CH], F32, tag="pg")
                for k in range(MH):
                    nc.tensor.matmul(psg, owt[:, k * P:(k + 1) * P],
                                     yft[:, k, :],
                                     start=(k == 0), stop=(k == MH - 1))
                ot = ot_pool.tile([P, CH], F32, tag="ot")
                nc.scalar.copy(out=ot, in_=psg)
                nc.sync.dma_start(out=outp[mo * P:(mo + 1) * P, cols(th)],
                                  in_=ot)

        def out_proj_stream(th):
            """Chunk-1 out_proj: per-set streamed accumulation. Six psg banks
            (4 from ps_a, idle after phase A, + 2 from ps_xg) accumulate the
            k-contractions as each gate set completes, so only one matmul per
            output tile trails the final gate."""
            yft = yf_tile(th)
            NS = 5  # five tiles streamed (3 ps_a + 2 ps_xg banks); the last
            ows, psgs = [], []
            for mo in range(NS):
                owt = ow_pool.tile([P, MH * P], BF16, tag="ow",
                                   name=f"owS{mo}")
                nc.sync.dma_start(out=owt, in_=owX[:, mo, :])
                pool = ps_a if mo < 3 else ps_xg
                tag = "ps" if mo < 3 else "pg"
                psgs.append(pool.tile([P, CH], F32, tag=tag, name=f"psg{mo}"))
                ows.append(owt)
            for s in range(len(SETS)):
                yield s
                for mo in range(NS):
                    for k in SETS[s]:
                        nc.tensor.matmul(psgs[mo][:, :],
                                         ows[mo][:, k * P:(k + 1) * P],
                                         yft[:, k, :],
                                         start=(k == 0), stop=(k == MH - 1))
            for mo in range(NS):
                ot = ot_pool.tile([P, CH], F32, tag="ot")
                nc.scalar.copy(out=ot, in_=psgs[mo])
                nc.sync.dma_start(out=outp[mo * P:(mo + 1) * P, cols(th)],
                                  in_=ot)
            for mo in range(NS, KM):
                owt = ow_pool.tile([P, MH * P], BF16, tag="ow")
                nc.sync.dma_start(out=owt, in_=owX[:, mo, :])
                psg = ps_a.tile([P, CH], F32, tag="ps")
                for k in range(MH):
                    nc.tensor.matmul(psg, owt[:, k * P:(k + 1) * P],
                                     yft[:, k, :],
                                     start=(k == 0), stop=(k == MH - 1))
                ot = ot_pool.tile([P, CH], F32, tag="ot")
                nc.scalar.copy(out=ot, in_=psg)
                nc.sync.dma_start(out=outp[mo * P:(mo + 1) * P, cols(th)],
                                  in_=ot)

        # ---------------- emission schedule ----------------
        state["psx"][0] = psx_tile("psx0")
        pend = None
        for m in range(MU):
            cur = (0, m, u_inproj(0, m))
            if pend is not None:
                (pth, pm, (pps, pdj)) = pend
                u_block(pth, pm, False, pps, pdj)
            pend = cur
            if m == 1:
                nc.sync.dma_start(out=dtw_sb, in_=dtwT[:, :])
                nc.sync.dma_start(out=A_sb, in_=Amat[:, :, :])
        (pth, pm, (pps, pdj)) = pend
        u_block(pth, pm, False, pps, pdj)
        for mz in range(MH):
            z_block(0, mz, defer=False)
            if mz == 0:
                nc.sync.dma_start(out=xs[:, :, CH:SEQ], in_=xT[:, :, CH:SEQ])
            elif mz == 2:
                nc.sync.dma_start(out=eye_sb, in_=eyeX[:, :])
                nc.sync.dma_start(out=dD_sb, in_=dDX[:, :])
        dt_softplus(0)
        bc_stage(0)
        w_mult(0)

        # chunk-1 projections (pre-silu) — PE/ACT-copy work that overlaps
        # the chunk-0 scan stream below
        pend = None
        for m in range(MU):
            cur = (1, m, u_inproj(1, m))
            if pend is not None:
                (pth, pm, (pps, pdj)) = pend
                u_block(pth, pm, True, pps, pdj)
            pend = cur
        (pth, pm, (pps, pdj)) = pend
        u_block(pth, pm, True, pps, pdj)
        for mz in range(MH):
            z_block(1, mz, defer=True)

        dA_set(0, 0)
        scan_set(0, 0)
        # scheduling fence: token is written once the first set of chunk-0
        # dA tiles exists, releasing the deferred silu batch below after the
        # first contiguous block of exps
        last_da = state[("da", 0, 0, NGRP - 1, SETS[0][-1])]
        nc.vector.tensor_scalar_mul(token, last_da[:, 0, 0:1], 0.0)

        silu_batch(1)
        dA_set(0, 1)
        state["psx"][1] = psx_tile("psx1")
        xproj_late(1)
        dt_softplus(1)
        bc_stage(1)

        scan_set(0, 1)
        w_mult(1)
        gates(0, 0)
        gates(0, 1)

        dA_set(1, 0)
        out_proj(0)
        scan_set(1, 0)
        ops = out_proj_stream(1)
        next(ops)
        dA_set(1, 1)
        scan_set(1, 1)
        gates(1, 0)
        next(ops)
        gates(1, 1)
        for _ in ops:
            pass

    nc.finalize()
    return nc


def _prep_core(x, prm, b, direction, half):
    """Build the per-core input map. prm maps param name -> array."""
    xb = np.ascontiguousarray(x[b])                # (L, D_MODEL)
    if direction == 1:
        xb = np.ascontiguousarray(xb[::-1])
    in_w = prm["in_w"]
    conv_w = prm["conv_w"]
    conv_b = prm["conv_b"]
    xproj_w = prm["xproj_w"]
    dt_w = prm["dt_w"]
    dt_b = prm["dt_b"]
    Alog = prm["Alog"]
    Dp = prm["D"]
    out_w = prm["out_w"]

    own = np.arange(half * DH, (half + 1) * DH)
    oth = np.arange((1 - half) * DH, (2 - half) * DH)
    perm = np.concatenate([own, oth])              # u-channel permutation

    wu = in_w[0:D_INNER][perm]                     # (1536, 768), own half first
    wz = in_w[D_INNER:2 * D_INNER][own]            # (768, 768)
    cw = conv_w[perm]                              # (1536, 4)
    A = -np.exp(Alog[own])                         # (768, 16)
    bf = ml_dtypes.bfloat16

    def lhs_tiles(mat_t, kk, mm):
        # (K*P, M*P) -> (mm, P, kk*P): per m-tile, partition-contiguous rows
        return np.ascontiguousarray(
            mat_t.reshape(kk, P, mm, P).transpose(2, 1, 0, 3).reshape(mm, P, kk * P))

    # conv taps as diagonal matmul weights: djX[m, p, j*P+q] = (p==q)*cw[mP+p, j]
    eye = np.eye(P, dtype=np.float32)
    dj = np.einsum("pq,mpj->mpjq", eye,
                   cw.reshape(MU, P, D_CONV)).reshape(MU, P, D_CONV * P)
    # D-skip diagonals: dDX[p, k*P+q] = (p==q)*D[kP+p]
    dD = np.einsum("pq,kp->pkq", eye,
                   Dp[own].reshape(MH, P)).reshape(P, MH * P)
    # out_proj: owX[p, mo, k*P+q] = out_w[mo*P+q, own[k*P+p]]
    ow = out_w[:, own].reshape(KM, P, MH, P).transpose(3, 0, 2, 1)  # p,mo,k,q
    ow = np.ascontiguousarray(ow.transpose(0, 1, 2, 3)).reshape(P, KM, MH * P)

    return {
        "xT": np.ascontiguousarray(xb.T.reshape(KM, P, SEQ).transpose(1, 0, 2)),
        "wuX": lhs_tiles(wu.T, KM, MU),
        "wzX": lhs_tiles(wz.T, KM, MH),
        "djX": dj.astype(bf),
        "dDX": dD.astype(bf),
        "eyeX": eye.astype(bf),
        "cbias": np.ascontiguousarray(conv_b[perm].reshape(MU, P).T),
        "xpX": np.ascontiguousarray(
            xproj_w[:, perm].T.reshape(MU, P, 80).transpose(1, 0, 2)).astype(bf),
        "dtwT": np.ascontiguousarray(
            np.vstack([dt_w[own].T, dt_b[own][None, :]])),
        "ones1": np.ones((1, CH), dtype=np.float32),
        "Amat": np.ascontiguousarray(A.reshape(MH, P, D_STATE).transpose(1, 0, 2)),
        "owX": np.ascontiguousarray(ow).astype(bf),
    }


def _in_maps(inputs):
    x = inputs["x"]
    maps = []
    for b in range(BATCH):
        for direction in range(2):
            pfx = "f" if direction == 0 else "b"
            prm = {k: inputs[f"{pfx}_{k}"] for k in
                   ("in_w", "conv_w", "conv_b", "xproj_w", "dt_w", "dt_b",
                    "Alog", "D", "out_w")}
            for half in range(2):
                maps.append(_prep_core(x, prm, b, direction, half))
    return maps


def kernel(**inputs):
    inputs = {k: np.asarray(v, dtype=np.float32) for k, v in inputs.items()}
    nc = _CACHE.get("nc")
    if nc is None:
        nc = _build()
        _CACHE["nc"] = nc
    maps = _in_maps(inputs)
    res = run_bass_kernel_spmd(nc, maps, list(range(8)),
                               **_CACHE.get("run_kwargs", {}))
    _CACHE["last_results"] = res
    out = np.zeros((BATCH, SEQ, D_MODEL), dtype=np.float32)
    ci = 0
    for b in range(BATCH):
        for direction in range(2):
            for half in range(2):
                part = res.results[ci]["outp"].T          # (SEQ, D_MODEL)
                if direction == 1:
                    part = part[::-1]
                out[b] += part
                ci += 1
    return out


# revision 61
# speedup vs baseline: 1.0040x; 1.0027x over previous
"""Bidirectional Mamba layer for Trainium2 (8 NeuronCores).

Sharding: core = (batch b in {0,1}) x (direction in {fwd,bwd}) x (d_inner half).
All 8 cores run one SPMD program with per-core input arrays; no collectives.
The host flips the sequence for the backward direction, permutes u-channels so
each core's own d_inner half is channel-tiles 0..5, and pre-builds every weight
layout (including the depthwise-conv taps and the D-skip as ready diagonal
matrices) so the engines never build operands at runtime.

v2: the sequence is processed in two 512-column chunks, software-pipelined so
the selective scan for chunk 0 runs while the tensor engine projects chunk 1.
Engine assignment per (d-tile, state-group): dA=exp(delta*A) on ACT, dbu and
the hardware tensor_tensor_scan on DVE (bf16 keeps dbu in the 2x DVE mode),
g = h*C mostly on the otherwise-idle GPSIMD engine, y = sum_n g as identity
matmuls accumulated in PSUM on PE, gating on DVE. Chunk-1 scans chain the
chunk-0 state via per-(d,n) carry columns and the scan's initial-AP operand.
"""
import sys

sys.path.insert(0, "/opt/trn_rl_repo")

from contextlib import ExitStack

import ml_dtypes
import numpy as np

import concourse.bass as bass
import concourse.mybir as mybir
import concourse.tile as tile
from concourse import bacc
from concourse.bass_utils import run_bass_kernel_spmd

D_MODEL = 768
D_STATE = 16
D_INNER = 1536
DT_RANK = 48
D_CONV = 4
BATCH = 2
SEQ = 1024
DH = D_INNER // 2          # 768 scan channels per core
P = 128
KM = D_MODEL // P          # 6 k-tiles over d_model
MU = D_INNER // P          # 12 m-tiles for full u
MH = DH // P               # 6 m-tiles for the own half
CH = 512                   # chunk width (2 chunks over SEQ)
NB = 4                     # states per scan group
NGRP = D_STATE // NB       # 4 groups
SP = CH + 2                # scan block width incl 2 zero/pad columns
SETS = ((0, 1, 2), (3, 4, 5))

F32 = mybir.dt.float32
F32R = mybir.dt.float32r
BF16 = mybir.dt.bfloat16
AF = mybir.ActivationFunctionType
OP = mybir.AluOpType

_CACHE = {}


def _build():
    nc = bacc.Bacc("TRN2", target_bir_lowering=False, debug=False)

    xT = nc.dram_tensor("xT", [P, KM, SEQ], F32R, kind="ExternalInput")
    wuX = nc.dram_tensor("wuX", [MU, P, KM * P], F32R, kind="ExternalInput")
    wzX = nc.dram_tensor("wzX", [MH, P, KM * P], F32R, kind="ExternalInput")
    djX = nc.dram_tensor("djX", [MU, P, D_CONV * P], BF16, kind="ExternalInput")
    dDX = nc.dram_tensor("dDX", [P, MH * P], BF16, kind="ExternalInput")
    eyeX = nc.dram_tensor("eyeX", [P, P], BF16, kind="ExternalInput")
    cbias = nc.dram_tensor("cbias", [P, MU], F32, kind="ExternalInput")
    xpX = nc.dram_tensor("xpX", [P, MU, 80], BF16, kind="ExternalInput")
    dtwT = nc.dram_tensor("dtwT", [DT_RANK + 1, DH], F32R, kind="ExternalInput")
    ones1 = nc.dram_tensor("ones1", [1, CH], F32R, kind="ExternalInput")
    Amat = nc.dram_tensor("Amat", [P, MH, D_STATE], F32, kind="ExternalInput")
    owX = nc.dram_tensor("owX", [P, KM, MH * P], BF16, kind="ExternalInput")
    outp = nc.dram_tensor("outp", [D_MODEL, SEQ], F32, kind="ExternalOutput")

    with tile.TileContext(nc) as tc, ExitStack() as top:
        persist = top.enter_context(tc.tile_pool(name="persist", bufs=1))
        xs_pool = top.enter_context(tc.tile_pool(name="xs", bufs=1))
        uoth_pool = top.enter_context(tc.tile_pool(name="uoth", bufs=6))
        wpool = top.enter_context(tc.tile_pool(name="wst", bufs=2))
        djpool = top.enter_context(tc.tile_pool(name="djst", bufs=2))
        ubuf_pool = top.enter_context(tc.tile_pool(name="ubuf", bufs=3))
        xdr_pool = top.enter_context(tc.tile_pool(name="xdr", bufs=2))
        xbc_pool = top.enter_context(tc.tile_pool(name="xbc", bufs=2))
        bcg_pool = top.enter_context(tc.tile_pool(name="bcg", bufs=2))
        da_pool = top.enter_context(tc.tile_pool(name="da", bufs=4))
        dbu_pool = top.enter_context(tc.tile_pool(name="dbu", bufs=4))
        h_pool = top.enter_context(tc.tile_pool(name="h", bufs=5))
        g_pool = top.enter_context(tc.tile_pool(name="g", bufs=5))
        yf_pool = top.enter_context(tc.tile_pool(name="yf", bufs=1))
        ot_pool = top.enter_context(tc.tile_pool(name="ot", bufs=2))
        ow_pool = top.enter_context(tc.tile_pool(name="owst", bufs=2))
        dram = top.enter_context(tc.tile_pool(name="dram", bufs=2, space="DRAM"))
        ps_a = top.enter_context(tc.tile_pool(name="ps_a", bufs=3, space="PSUM"))
        ps_xg = top.enter_context(tc.tile_pool(name="ps_xg", bufs=2, space="PSUM"))
        ps_y = top.enter_context(tc.tile_pool(name="ps_y", bufs=3, space="PSUM"))

        u_own = persist.tile([P, MH, SEQ], BF16, tag="uown")
        sz = persist.tile([P, MH, SEQ], BF16, tag="sz")
        delta = persist.tile([P, MH, SEQ], BF16, tag="dl")
        wdu = persist.tile([P, MH, SEQ], BF16, tag="wdu")
        carry = persist.tile([P, MH, D_STATE], BF16, tag="carry")
        A_sb = persist.tile([P, MH, D_STATE], F32, tag="A")
        cb_sb = persist.tile([P, MU], F32, tag="cb")
        dtw_sb = persist.tile([DT_RANK + 1, DH], F32R, tag="dtw")
        eye_sb = persist.tile([P, P], BF16, tag="eye")
        dD_sb = persist.tile([P, MH * P], BF16, tag="dD")
        xp_sb = persist.tile([P, MU, 80], BF16, tag="xp")
        halo = persist.tile([P, MU, 3], BF16, tag="halo")
        token = persist.tile([P, 1], BF16, tag="tok")
        one3 = persist.tile([P, 3], BF16, tag="one3")
        xs = xs_pool.tile([P, KM, SEQ], F32R, tag="xs")

        # first chunk of x + the first weight tiles lead the DMA queue so the
        # tensor engine starts as early as possible; bulk loads follow later
        nc.sync.dma_start(out=xs[:, :, 0:CH], in_=xT[:, :, 0:CH])
        nc.sync.dma_start(out=cb_sb, in_=cbias[:, :])
        nc.gpsimd.memset(one3, 1.0)
        nc.sync.dma_start(out=xp_sb, in_=xpX[:, :, :])

        state = {"ubuf_n": 0, "da_n": 0, "dbu_n": 0,
                 "uref": {}, "psx": {}, "yps": {}, "bcd": {}, "yf": {}}

        def cols(th):
            return slice(th * CH, (th + 1) * CH)

        # ---------------- phase A building blocks ----------------
        def psx_tile(name):
            t = ps_xg.tile([P, CH], F32, tag="pg", name=name)
            return t[0:80, :]

        def u_inproj(th, m):
            wu_m = wpool.tile([P, KM * P], F32R, tag="w")
            nc.sync.dma_start(out=wu_m, in_=wuX[m, :, :])
            dj = djpool.tile([P, D_CONV * P], BF16, tag="dj")
            nc.sync.dma_start(out=dj, in_=djX[m, :, :])
            ps = ps_a.tile([P, CH], F32, tag="ps")
            for k in range(KM):
                nc.tensor.matmul(ps, wu_m[:, k * P:(k + 1) * P],
                                 xs[:, k, cols(th)],
                                 start=(k == 0), stop=(k == KM - 1))
            return ps, dj

        def u_block(th, m, defer, ps, dj):
            """causal conv -> (silu or deferred) u tile, plus the xproj
            contribution when not deferred."""
            ub = ubuf_pool.tile([P, 3 + CH], BF16, tag="ub")
            if th == 0:
                if state["ubuf_n"] < 3:
                    nc.gpsimd.memset(ub[:, 0:3], 0.0)
                state["ubuf_n"] += 1
            else:
                nc.gpsimd.tensor_tensor(out=ub[:, 0:3], in0=halo[:, m, :],
                                        in1=one3, op=OP.mult)
            if th == 0:
                nc.scalar.copy(out=ub[:, 3:3 + CH], in_=ps)
                nc.gpsimd.tensor_tensor(out=halo[:, m, :], in0=ub[:, CH:CH + 3],
                                        in1=one3, op=OP.mult)
            else:
                # chunk-1 staging on DVE: lands in the scan-stream troughs and
                # unloads the oversubscribed ACT transition window
                nc.vector.tensor_scalar_mul(ub[:, 3:3 + CH], ps, 1.0)
            psc = ps_a.tile([P, CH], F32, tag="ps")
            for j in range(D_CONV):
                nc.tensor.matmul(psc, dj[:, j * P:(j + 1) * P],
                                 ub[:, j:j + CH],
                                 start=(j == 0), stop=(j == D_CONV - 1))
            if m < MH:
                dest = u_own[:, m, cols(th)]
            else:
                dest = uoth_pool.tile([P, CH], BF16, tag="uo", name=f"uo{th}_{m}")
            if not defer:
                nc.scalar.activation(out=dest, in_=psc, func=AF.Silu,
                                     bias=cb_sb[:, m:m + 1])
                nc.tensor.matmul(state["psx"][th], xp_sb[:, m, :], dest,
                                 start=(m == 0), stop=(m == MU - 1))
            else:
                nc.scalar.activation(out=dest, in_=psc, func=AF.Identity,
                                     bias=cb_sb[:, m:m + 1])
            state["uref"][(th, m)] = dest

        def z_block(th, mz, defer):
            wz_m = wpool.tile([P, KM * P], F32R, tag="w")
            nc.sync.dma_start(out=wz_m, in_=wzX[mz, :, :])
            ps = ps_a.tile([P, CH], F32, tag="ps")
            for k in range(KM):
                nc.tensor.matmul(ps, wz_m[:, k * P:(k + 1) * P],
                                 xs[:, k, cols(th)],
                                 start=(k == 0), stop=(k == KM - 1))
            if not defer:
                nc.scalar.activation(out=sz[:, mz, cols(th)], in_=ps, func=AF.Silu)
            else:
                nc.vector.tensor_scalar_mul(sz[:, mz, cols(th)], ps, 1.0)

        def silu_batch(th):
            """Deferred in-place silus for chunk th (u own, u other, z).
            The zero `token` bias is a scheduling fence: the greedy per-engine
            scheduler would otherwise hoist these silus into idle slots of the
            chunk-0 dA exp stream, thrashing the ACT function table (silu and
            exp share no table). The token is produced only after the last
            chunk-0 dA tile, so these stay one contiguous batch."""
            for m in range(MU):
                dest = state["uref"][(th, m)]
                nc.scalar.activation(out=dest, in_=dest, func=AF.Silu,
                                     bias=token[:, 0:1])
            for mz in range(MH):
                s = sz[:, mz, cols(th)]
                nc.scalar.activation(out=s, in_=s, func=AF.Silu,
                                     bias=token[:, 0:1])

        def xproj_late(th):
            for m in range(MU):
                nc.tensor.matmul(state["psx"][th], xp_sb[:, m, :],
                                 state["uref"][(th, m)],
                                 start=(m == 0), stop=(m == MU - 1))

        def dt_softplus(th):
            psx = state["psx"][th]
            xdr = xdr_pool.tile([64, CH], F32R, tag="xdr")
            nc.scalar.copy(out=xdr[0:32, :], in_=psx[0:32, :])
            nc.scalar.copy(out=xdr[32:64, :], in_=psx[32:64, :])
            nc.scalar.dma_start(out=xdr[DT_RANK:DT_RANK + 1, :],
                                in_=ones1[:, :])
            dcol = delta[:, :, cols(th)]
            for m in range(MH):
                psd = ps_a.tile([P, CH], F32, tag="ps")
                nc.tensor.matmul(psd, dtw_sb[:, m * P:(m + 1) * P],
                                 xdr[0:DT_RANK + 1, :], start=True, stop=True)
                nc.scalar.activation(out=delta[:, m, cols(th)], in_=psd,
                                     func=AF.Exp)
            # softplus tail: delta = ln(exp(.) + 1), computed in place
            nc.scalar.activation(out=dcol, in_=dcol, func=AF.Ln, bias=1.0)

        def bc_stage(th):
            psx = state["psx"][th]
            xbc = xbc_pool.tile([48, CH], BF16, tag="xbc")
            nc.scalar.copy(out=xbc[0:32, :], in_=psx[32:64, :])
            nc.scalar.copy(out=xbc[32:48, :], in_=psx[64:80, :])
            bcd = dram.tile([2 * D_STATE, CH], BF16, tag="bcd")
            nc.scalar.dma_start(out=bcd, in_=xbc[16:48, :])
            state["bcd"][th] = bcd

        def w_mult(th):
            for m in range(MH):
                nc.vector.tensor_tensor(out=wdu[:, m, cols(th)],
                                        in0=delta[:, m, cols(th)],
                                        in1=u_own[:, m, cols(th)], op=OP.mult)

        # ---------------- phase B: scans ----------------
        def dA_set(th, s):
            """dA for one d-tile set. Groups 0-1 (n=0..7) are exps on ACT;
            groups 2-3 reuse them as DVE bf16 products: q^(8+k) = q^8*q^k
            (A is the S4D-real init, so dA_n = exp(-(n+1)*delta) = q^(n+1)).
            The da pool holds a full set so product sources stay live."""
            for ng in range(NGRP):
                for m in SETS[s]:
                    dat = da_pool.tile([P, NB, SP], BF16, tag="da")
                    if state["da_n"] < 4:
                        nc.gpsimd.memset(dat[:, :, CH:SP], 0.0)
                    state["da_n"] += 1
                    for j in range(NB):
                        n = ng * NB + j
                        nc.scalar.activation(out=dat[:, j, 0:CH],
                                             in_=delta[:, m, cols(th)],
                                             func=AF.Exp,
                                             scale=A_sb[:, m, n:n + 1])
                    state[("da", th, s, ng, m)] = dat

        def scan_set(th, s):
            """One set of 3 d-tiles: all 4 state-groups, scans + g + yacc."""
            gt_ref = {}
            yps = {m: ps_y.tile([P, CH], F32, tag="yps", name=f"yps{th}{s}{m}")
                   for m in SETS[s]}
            state["yps"].update({(th, m): yps[m] for m in SETS[s]})
            for ng in range(NGRP):
                bcgt = bcg_pool.tile([P, 2, NB, CH], BF16, tag="bcg")
                src = bass.AP(
                    tensor=state["bcd"][th].tensor,
                    offset=state["bcd"][th].offset + ng * NB * CH,
                    ap=[[0, P], [D_STATE * CH, 2], [CH, NB], [1, CH]])
                nc.scalar.dma_start(out=bcgt, in_=src)
                for m in SETS[s]:
                    dat = state[("da", th, s, ng, m)]
                    dbut = dbu_pool.tile([P, NB, SP], BF16, tag="dbu")
                    if state["dbu_n"] < 4:
                        nc.gpsimd.memset(dbut[:, :, CH:SP], 0.0)
                    state["dbu_n"] += 1
                    nc.vector.tensor_tensor(
                        out=dbut[:, :, 0:CH],
                        in0=wdu[:, m, cols(th)].unsqueeze(1)
                            .broadcast_to([P, NB, CH]),
                        in1=bcgt[:, 0, :, :], op=OP.mult)
                    ht = h_pool.tile([P, NB, SP], BF16, tag="h")
                    if th == 0:
                        nc.vector.tensor_tensor_scan(
                            out=ht.rearrange("p a b -> p (a b)"),
                            data0=dat.rearrange("p a b -> p (a b)"),
                            data1=dbut.rearrange("p a b -> p (a b)"),
                            initial=0.0, op0=OP.mult, op1=OP.add)
                        nc.vector.tensor_scalar_mul(
                            carry[:, m, ng * NB:(ng + 1) * NB],
                            ht[:, :, CH - 1:CH].rearrange("p a b -> p (a b)"),
                            1.0)
                    else:
                        for j in range(NB):
                            n = ng * NB + j
                            nc.vector.tensor_tensor_scan(
                                out=ht[:, j, 0:CH], data0=dat[:, j, 0:CH],
                                data1=dbut[:, j, 0:CH],
                                initial=carry[:, m, n:n + 1],
                                op0=OP.mult, op1=OP.add)
                    gt = g_pool.tile([P, NB, CH], BF16, tag="g")
                    # g = h*C split 3:1 between GPSIMD and DVE so neither
                    # paces the chunk pipeline alone
                    nc.gpsimd.tensor_tensor(out=gt[:, 0:3, :],
                                            in0=ht[:, 0:3, 0:CH],
                                            in1=bcgt[:, 1, 0:3, :], op=OP.mult)
                    nc.vector.tensor_tensor(out=gt[:, 3, :],
                                            in0=ht[:, 3, 0:CH],
                                            in1=bcgt[:, 1, 3, :], op=OP.mult)
                    gt_ref[(m, ng)] = gt
                for m in SETS[s]:
                    for j in range(NB):
                        nc.tensor.matmul(yps[m][:, :], eye_sb,
                                         gt_ref[(m, ng)][:, j, :],
                                         start=(ng == 0 and j == 0), stop=False)
            for m in SETS[s]:
                nc.tensor.matmul(yps[m][:, :], dD_sb[:, m * P:(m + 1) * P],
                                 u_own[:, m, cols(th)], start=False, stop=True)

        def yf_tile(th):
            yft = state["yf"].get(th)
            if yft is None:
                yft = yf_pool.tile([P, MH, CH], BF16, tag="yf", name=f"yf{th}")
                state["yf"][th] = yft
            return yft

        def gates(th, s):
            yft = yf_tile(th)
            for m in SETS[s]:
                nc.vector.tensor_tensor(out=yft[:, m, :],
                                        in0=state["yps"][(th, m)],
                                        in1=sz[:, m, cols(th)], op=OP.mult)

        def out_proj(th):
            yft = state["yf"][th]
            for mo in range(KM):
                owt = ow_pool.tile([P, MH * P], BF16, tag="ow")
                nc.sync.dma_start(out=owt, in_=owX[:, mo, :])
                psg = ps_xg.tile([P, CH], F32, tag="pg")
                for k in range(MH):
                    nc.tensor.matmul(psg, owt[:, k * P:(k + 1) * P],
                                     yft[:, k, :],
                                     start=(k == 0), stop=(k == MH - 1))
                ot = ot_pool.tile([P, CH], F32, tag="ot")
                nc.scalar.copy(out=ot, in_=psg)
                nc.sync.dma_start(out=outp[mo * P:(mo + 1) * P, cols(th)],
                                  in_=ot)

        def out_proj_stream(th):
            """Chunk-1 out_proj: per-set streamed accumulation. Six psg banks
            (4 from ps_a, idle after phase A, + 2 from ps_xg) accumulate the
            k-contractions as each gate set completes, so only one matmul per
            output tile trails the final gate."""
            yft = yf_tile(th)
            NS = 5  # five tiles streamed (3 ps_a + 2 ps_xg banks); the last
            ows, psgs = [], []
            for mo in range(NS):
                owt = ow_pool.tile([P, MH * P], BF16, tag="ow",
                                   name=f"owS{mo}")
                nc.sync.dma_start(out=owt, in_=owX[:, mo, :])
                pool = ps_a if mo < 3 else ps_xg
                tag = "ps" if mo < 3 else "pg"
                psgs.append(pool.tile([P, CH], F32, tag=tag, name=f"psg{mo}"))
                ows.append(owt)
            for s in range(len(SETS)):
                yield s
                for mo in range(NS):
                    for k in SETS[s]:
                        nc.tensor.matmul(psgs[mo][:, :],
                                         ows[mo][:, k * P:(k + 1) * P],
                                         yft[:, k, :],
                                         start=(k == 0), stop=(k == MH - 1))
            for mo in range(NS):
                ot = ot_pool.tile([P, CH], F32, tag="ot")
                nc.scalar.copy(out=ot, in_=psgs[mo])
                nc.sync.dma_start(out=outp[mo * P:(mo + 1) * P, cols(th)],
                                  in_=ot)
            for mo in range(NS, KM):
                owt = ow_pool.tile([P, MH * P], BF16, tag="ow")
                nc.sync.dma_start(out=owt, in_=owX[:, mo, :])
                psg = ps_a.tile([P, CH], F32, tag="ps")
                for k in range(MH):
                    nc.tensor.matmul(psg, owt[:, k * P:(k + 1) * P],
                                     yft[:, k, :],
                                     start=(k == 0), stop=(k == MH - 1))
                ot = ot_pool.tile([P, CH], F32, tag="ot")
                nc.scalar.copy(out=ot, in_=psg)
                nc.sync.dma_start(out=outp[mo * P:(mo + 1) * P, cols(th)],
                                  in_=ot)

        # ---------------- emission schedule ----------------
        state["psx"][0] = psx_tile("psx0")
        pend = None
        for m in range(MU):
            cur = (0, m, u_inproj(0, m))
            if pend is not None:
                (pth, pm, (pps, pdj)) = pend
                u_block(pth, pm, False, pps, pdj)
            pend = cur
            if m == 1:
                nc.sync.dma_start(out=dtw_sb, in_=dtwT[:, :])
                nc.sync.dma_start(out=A_sb, in_=Amat[:, :, :])
        (pth, pm, (pps, pdj)) = pend
        u_block(pth, pm, False, pps, pdj)
        for mz in range(MH):
            z_block(0, mz, defer=False)
            if mz == 0:
                nc.sync.dma_start(out=xs[:, :, CH:SEQ], in_=xT[:, :, CH:SEQ])
            elif mz == 2:
                nc.sync.dma_start(out=eye_sb, in_=eyeX[:, :])
                nc.sync.dma_start(out=dD_sb, in_=dDX[:, :])
        dt_softplus(0)
        bc_stage(0)
        w_mult(0)

        # chunk-1 projections (pre-silu) — PE/ACT-copy work that overlaps
        # the chunk-0 scan stream below
        pend = None
        for m in range(MU):
            cur = (1, m, u_inproj(1, m))
            if pend is not None:
                (pth, pm, (pps, pdj)) = pend
                u_block(pth, pm, True, pps, pdj)
            pend = cur
        (pth, pm, (pps, pdj)) = pend
        u_block(pth, pm, True, pps, pdj)

        dA_set(0, 0)
        scan_set(0, 0)
        # chunk-1 z after the first scan set: its matmuls are ready early but
        # must not outrank the scan-critical PE work in the priority order
        # (still emitted before silu_batch, which silus sz in place)
        for mz in range(MH):
            z_block(1, mz, defer=True)
        # scheduling fence: token is written once the first set of chunk-0
        # dA tiles exists, releasing the deferred silu batch below after the
        # first contiguous block of exps
        last_da = state[("da", 0, 0, NGRP - 1, SETS[0][-1])]
        nc.vector.tensor_scalar_mul(token, last_da[:, 0, 0:1], 0.0)

        silu_batch(1)
        dA_set(0, 1)
        state["psx"][1] = psx_tile("psx1")
        xproj_late(1)
        dt_softplus(1)
        bc_stage(1)

        scan_set(0, 1)
        w_mult(1)
        gates(0, 0)
        gates(0, 1)

        dA_set(1, 0)
        out_proj(0)
        scan_set(1, 0)
        ops = out_proj_stream(1)
        next(ops)
        dA_set(1, 1)
        scan_set(1, 1)
        gates(1, 0)
        next(ops)
        gates(1, 1)
        for _ in ops:
            pass

    nc.finalize()
    return nc


def _prep_core(x, prm, b, direction, half):
    """Build the per-core input map. prm maps param name -> array."""
    xb = np.ascontiguousarray(x[b])                # (L, D_MODEL)
    if direction == 1:
        xb = np.ascontiguousarray(xb[::-1])
    in_w = prm["in_w"]
    conv_w = prm["conv_w"]
    conv_b = prm["conv_b"]
    xproj_w = prm["xproj_w"]
    dt_w = prm["dt_w"]
    dt_b = prm["dt_b"]
    Alog = prm["Alog"]
    Dp = prm["D"]
    out_w = prm["out_w"]

    own = np.arange(half * DH, (half + 1) * DH)
    oth = np.arange((1 - half) * DH, (2 - half) * DH)
    perm = np.concatenate([own, oth])              # u-channel permutation

    wu = in_w[0:D_INNER][perm]                     # (1536, 768), own half first
    wz = in_w[D_INNER:2 * D_INNER][own]            # (768, 768)
    cw = conv_w[perm]                              # (1536, 4)
    A = -np.exp(Alog[own])                         # (768, 16)
    bf = ml_dtypes.bfloat16

    def lhs_tiles(mat_t, kk, mm):
        # (K*P, M*P) -> (mm, P, kk*P): per m-tile, partition-contiguous rows
        return np.ascontiguousarray(
            mat_t.reshape(kk, P, mm, P).transpose(2, 1, 0, 3).reshape(mm, P, kk * P))

    # conv taps as diagonal matmul weights: djX[m, p, j*P+q] = (p==q)*cw[mP+p, j]
    eye = np.eye(P, dtype=np.float32)
    dj = np.einsum("pq,mpj->mpjq", eye,
                   cw.reshape(MU, P, D_CONV)).reshape(MU, P, D_CONV * P)
    # D-skip diagonals: dDX[p, k*P+q] = (p==q)*D[kP+p]
    dD = np.einsum("pq,kp->pkq", eye,
                   Dp[own].reshape(MH, P)).reshape(P, MH * P)
    # out_proj: owX[p, mo, k*P+q] = out_w[mo*P+q, own[k*P+p]]
    ow = out_w[:, own].reshape(KM, P, MH, P).transpose(3, 0, 2, 1)  # p,mo,k,q
    ow = np.ascontiguousarray(ow.transpose(0, 1, 2, 3)).reshape(P, KM, MH * P)

    return {
        "xT": np.ascontiguousarray(xb.T.reshape(KM, P, SEQ).transpose(1, 0, 2)),
        "wuX": lhs_tiles(wu.T, KM, MU),
        "wzX": lhs_tiles(wz.T, KM, MH),
        "djX": dj.astype(bf),
        "dDX": dD.astype(bf),
        "eyeX": eye.astype(bf),
        "cbias": np.ascontiguousarray(conv_b[perm].reshape(MU, P).T),
        "xpX": np.ascontiguousarray(
            xproj_w[:, perm].T.reshape(MU, P, 80).transpose(1, 0, 2)).astype(bf),
        "dtwT": np.ascontiguousarray(
            np.vstack([dt_w[own].T, dt_b[own][None, :]])),
        "ones1": np.ones((1, CH), dtype=np.float32),
        "Amat": np.ascontiguousarray(A.reshape(MH, P, D_STATE).transpose(1, 0, 2)),
        "owX": np.ascontiguousarray(ow).astype(bf),
    }


def _in_maps(inputs):
    x = inputs["x"]
    maps = []
    for b in range(BATCH):
        for direction in range(2):
            pfx = "f" if direction == 0 else "b"
            prm = {k: inputs[f"{pfx}_{k}"] for k in
                   ("in_w", "conv_w", "conv_b", "xproj_w", "dt_w", "dt_b",
                    "Alog", "D", "out_w")}
            for half in range(2):
                maps.append(_prep_core(x, prm, b, direction, half))
    return maps


def kernel(**inputs):
    inputs = {k: np.asarray(v, dtype=np.float32) for k, v in inputs.items()}
    nc = _CACHE.get("nc")
    if nc is None:
        nc = _build()
        _CACHE["nc"] = nc
    maps = _in_maps(inputs)
    res = run_bass_kernel_spmd(nc, maps, list(range(8)),
                               **_CACHE.get("run_kwargs", {}))
    _CACHE["last_results"] = res
    out = np.zeros((BATCH, SEQ, D_MODEL), dtype=np.float32)
    ci = 0
    for b in range(BATCH):
        for direction in range(2):
            for half in range(2):
                part = res.results[ci]["outp"].T          # (SEQ, D_MODEL)
                if direction == 1:
                    part = part[::-1]
                out[b] += part
                ci += 1
    return out
